# revision 43
# baseline (speedup 1.0000x reference)
"""EMA recurrence kernel for Trainium2 (8 NeuronCores, Bass/Tile).

Computes a_t = w * x_t + (1 - w) * a_{t-1} over inputs [B=32, T=8192, C=128],
initial_state [B, C], weights [C] -> output [B, T, C].

Strategy (v7 -- depth-8 decimated scan, Q-space, uint8 output):
  - Pure data parallelism: batch dim sharded 4-per-core across 8 cores.
  - Everything on-device runs in "Q-space": host pre-scales v = (w*x)/s
    (fp16) where s = max|a|/126, so outputs quantize to uint8 via
    trunc(Q + 128.5) -- always positive, so truncation == floor == exact
    round-half-up regardless of HW convert semantics. Output HBM traffic
    is 1/4 of fp32, input traffic 1/2.
  - Time decimated by D=8 into streams d=0..7 (t = 8j + d):
      PE    U[j] = sum_d c^{7-d} v_d[j]  (8 diag-matmul passes -> PSUM,
            one PSUM tile per batch so each scan is fenced only on its
            own 8 matmuls -- the framework fences at tile granularity)
      DVE   scan Q8[j] = c^8 Q8[j-1] + U[j], reading U directly from
            PSUM (no ACT evacuation), fp16 out
      DVE   recon chain Y_0 = c*Q8[j-1] + v_0; Y_d = c*Y_{d-1} + v_d
            as TS (4x perf mode) + TT (2x perf mode), fused over the 2
            batches of a pair; one yd tile per stream so each ACT quant
            starts right after its own TT
      ACT   quantize fp16 -> uint8 (ACT cost is dtype-independent, so
            the int8 conversion is free there)
      DMA   input quarters on the SP ring; outputs on the GPSIMD ring
            (separate rings: both are in-order, so mixing input and
            output transfers head-of-line-blocks the pipeline)
  - The recon chain stays entirely on DVE: it is latency-sensitive and
    cross-engine chain steps (ACT or GPSIMD) measured worse. GPSIMD
    compute also degrades DVE throughput via SBUF arbitration.
  - Host-side DRAM layouts are partition-row contiguous ([C, cols]) so
    every DMA is a plain 2D copy (128 descriptors, KBs each).
  - Work units = (batch pair) x (column chunk): 4 units pipeline
    DMA/PE/DVE/ACT; measured best among pair/quad x LC {512, 1024}.
"""

import sys

if "/opt/trn_rl_repo" not in sys.path:
    sys.path.insert(0, "/opt/trn_rl_repo")

import numpy as np

B, T, C = 32, 8192, 128
NCORES = 8
BL = B // NCORES      # batches per core (4)
D = 8                 # decimation depth
L = T // D            # decimated stream length (1024)
NP = BL // 2          # batch pairs per core (2)
LC = 512              # scan chunk columns
KC = L // LC          # chunks per stream (2)
G = 2 * D             # blocks per unit: (half i, stream d), i-major
MM = 512              # matmul slice (PE moving-dim limit / PSUM bank)

_NC_CACHE = None


def build_bass():
    global _NC_CACHE
    if _NC_CACHE is not None:
        return _NC_CACHE

    import concourse.bacc as bacc
    import concourse.mybir as mybir
    import concourse.tile as tile

    f32 = mybir.dt.float32
    f16 = mybir.dt.float16
    u8 = mybir.dt.uint8
    i8 = mybir.dt.int8
    AF = mybir.ActivationFunctionType
    ALU = mybir.AluOpType

    W2 = 2 * LC           # fused pair width per stream (1024)
    VW = G * LC           # full unit width (8192)
    HW = D * LC           # half-unit width (one batch, 4096)
    SL = 1 + L            # per-batch scan row incl. init col

    nc = bacc.Bacc("TRN2", target_bir_lowering=False, debug=False)
    vin = nc.dram_tensor("vin", [NP, KC, C, VW], f16, kind="ExternalInput").ap()
    s0q = nc.dram_tensor("s0q", [C, BL], f16, kind="ExternalInput").ap()
    wkT = nc.dram_tensor("wkT", [C, D * 128], f16, kind="ExternalInput").ap()
    c8col = nc.dram_tensor("c8col", [C, 1], f32, kind="ExternalInput").ap()
    ccol = nc.dram_tensor("ccol", [C, 1], f32, kind="ExternalInput").ap()
    yq = nc.dram_tensor("yq", [NP, KC, C, VW], u8, kind="ExternalOutput").ap()

    def blk(i, d):        # vt/qt column base of block (half i, stream d)
        return (i * D + d) * LC

    with tile.TileContext(nc) as tc:
        with (
            tc.tile_pool(name="const", bufs=1) as cpool,
            tc.tile_pool(name="vin", bufs=4) as vpool,
            tc.tile_pool(name="ups", bufs=4, space="PSUM") as ppool,
            tc.tile_pool(name="y8", bufs=1) as spool,
            tc.tile_pool(name="work", bufs=3) as wpool,
            tc.tile_pool(name="yout", bufs=3) as ypool,
        ):
            # consts ride the ACT ring; the v stream starts at once on SP
            wkT_t = cpool.tile([C, D * 128], f16, name="wkT_t")
            nc.scalar.dma_start(wkT_t[:], wkT[:])
            c8_t = cpool.tile([C, 1], f32, name="c8_t")
            nc.scalar.dma_start(c8_t[:], c8col[:])
            c_t = cpool.tile([C, 1], f32, name="c_t")
            nc.scalar.dma_start(c_t[:], ccol[:])

            # pair-fused scan rows: batch i of pair p at cols [i*SL, i*SL+SL);
            # init cols via one tiny strided DMA per pair
            y8t = [spool.tile([C, 2 * SL], f16, name=f"y8_{p}") for p in range(NP)]
            for p in range(NP):
                nc.scalar.dma_start(
                    y8t[p][:].rearrange("c (i e) -> c i e", i=2)[:, :, 0:1],
                    s0q[:, 2 * p : 2 * p + 2].unsqueeze(2),
                )

            for k in range(KC):
                for p in range(NP):
                    b0 = 2 * p
                    lo, hi = k * LC, (k + 1) * LC

                    # ---- input DMA: quarter transfers (half i, streams
                    # d0..3 / d4..7) on SP so PE starts on the first quarter;
                    # vpool bufs=4 covers all units so these never wait
                    vt = vpool.tile([C, VW], f16, name=f"v{p}_{k}", tag="v")
                    QW = VW // 4
                    for q in range(4):
                        nc.sync.dma_start(
                            vt[:, q * QW : (q + 1) * QW],
                            vin[p][k][:, q * QW : (q + 1) * QW],
                        )

                    # ---- PE: U_i = sum_d diag(c^{7-d}) @ v_(i,d) into a
                    # per-batch PSUM tile; scan issues right after each
                    for i in range(2):
                        upi = ppool.tile([C, LC], f32, name=f"up{i}",
                                         tag=f"up{i}")
                        for d in range(D):
                            vcol = blk(i, d)
                            nc.tensor.matmul(
                                upi[:],
                                wkT_t[:, d * 128 : (d + 1) * 128],
                                vt[:, vcol : vcol + LC],
                                start=(d == 0),
                                stop=(d == D - 1),
                            )
                        base = i * SL
                        nc.vector.tensor_tensor_scan(
                            y8t[p][:, base + 1 + lo : base + 1 + hi],
                            c8_t[:, 0:1].broadcast_to([C, LC]),
                            upi[:],
                            y8t[p][:, base + lo : base + lo + 1],
                            op0=ALU.mult,
                            op1=ALU.add,
                        )

                    # ---- recon chain on DVE (TS 4x + TT 2x), pair-fused,
                    # one yd tile per stream for tile-granular fencing
                    qt = ypool.tile([C, VW], u8, name=f"q{p}_{k}", tag="q")
                    # scan stream (d = D-1) quantize first: it only needs the
                    # scans, so it runs while DVE begins the recon chain
                    nc.scalar.activation(
                        qt[:].rearrange("c (i g) -> c i g", i=2)[
                            :, :, (D - 1) * LC : D * LC
                        ],
                        y8t[p][:].rearrange("c (i e) -> c i e", i=2)[
                            :, :, 1 + lo : 1 + hi
                        ],
                        AF.Copy,
                        bias=128.5,
                        scale=1.0,
                    )
                    prev = None
                    for d in range(D - 2):
                        cae = wpool.tile([C, W2], f16, name="cae", tag="cae")
                        if d == 0 and p == 0 and k == 0:
                            # first unit: per-batch TS halves so DVE can run
                            # TS(i0) in the gap while PE finishes batch i1
                            for i in range(2):
                                nc.vector.tensor_scalar(
                                    cae[:, i * LC : (i + 1) * LC],
                                    y8t[p][:, i * SL + lo : i * SL + hi],
                                    c_t[:, 0:1],
                                    None,
                                    op0=ALU.mult,
                                )
                        elif d == 0:
                            src = y8t[p][:].rearrange("c (i e) -> c i e", i=2)[
                                :, :, lo:hi
                            ]
                            nc.vector.tensor_scalar(
                                cae[:].rearrange("c (i b) -> c i b", i=2),
                                src,
                                c_t[:, 0:1],
                                None,
                                op0=ALU.mult,
                            )
                        else:
                            nc.vector.tensor_scalar(
                                cae[:], prev[:], c_t[:, 0:1], None,
                                op0=ALU.mult,
                            )
                        v3 = vt[:].rearrange("c (i g) -> c i g", i=2)[
                            :, :, d * LC : (d + 1) * LC
                        ]
                        yd = wpool.tile([C, W2], f16, name=f"yd{d}",
                                        tag=f"yd{d}")
                        nc.vector.tensor_tensor(
                            yd[:].rearrange("c (i b) -> c i b", i=2),
                            cae[:].rearrange("c (i b) -> c i b", i=2),
                            v3,
                            op=ALU.add,
                        )
                        prev = yd
                        # quantize this stream now (starts right after TT)
                        nc.scalar.activation(
                            qt[:].rearrange("c (i g) -> c i g", i=2)[
                                :, :, d * LC : (d + 1) * LC
                            ],
                            yd[:].rearrange("c (i b) -> c i b", i=2),
                            AF.Copy,
                            bias=128.5,
                            scale=1.0,
                        )

                    # stream 6: chain step + quantize fused into ONE DVE STT
                    # writing signed int8 (y_6 has no downstream consumer, so
                    # the fp16 intermediate and its ACT quant are unnecessary)
                    nc.vector.scalar_tensor_tensor(
                        qt[:].rearrange("c (i g) -> c i g", i=2)[
                            :, :, (D - 2) * LC : (D - 1) * LC
                        ].bitcast(i8),
                        prev[:].rearrange("c (i b) -> c i b", i=2),
                        c_t[:, 0:1],
                        vt[:].rearrange("c (i g) -> c i g", i=2)[
                            :, :, (D - 2) * LC : (D - 1) * LC
                        ],
                        op0=ALU.mult,
                        op1=ALU.add,
                    )


                    # ---- output DMA quarters on the GPSIMD ring
                    OW = VW // 4
                    for h in range(4):
                        nc.gpsimd.dma_start(
                            yq[p][k][:, h * OW : (h + 1) * OW],
                            qt[:, h * OW : (h + 1) * OW],
                        )

    nc.compile()
    _NC_CACHE = nc
    return nc


def _prep(inputs, initial_state, weights):
    x = np.asarray(inputs, dtype=np.float32)
    s0 = np.asarray(initial_state, dtype=np.float32)
    w = np.clip(np.asarray(weights, dtype=np.float32), 0.0, 1.0)
    c = (1.0 - w).astype(np.float32)

    M = max(np.abs(x).max(), np.abs(s0).max())
    s = np.float32(M / 126.0)

    # v[b, j, d, ch] = w * x[b, 8j+d, ch] / s   (fp16)
    v = (w[None, None, :] * x / s).astype(np.float16)        # [B, T, C]
    v = v.reshape(B, L, D, C)

    s0q = (s0 / s).astype(np.float16)                        # [B, C]

    wkT = np.zeros((C, D * 128), np.float16)
    cd = c.astype(np.float64)
    for d in range(D):
        np.fill_diagonal(
            wkT[:, d * 128 : (d + 1) * 128], (cd ** (D - 1 - d)).astype(np.float16)
        )

    c8col = np.ascontiguousarray((cd**D).astype(np.float32)[:, None])
    ccol = np.ascontiguousarray(c[:, None])

    maps = []
    for core in range(NCORES):
        vb = v[core * BL : (core + 1) * BL]                  # [BL, L, D, C]
        vb = vb.reshape(NP, 2, KC, LC, D, C)                 # [p, i, k, jj, d, ch]
        vb = vb.transpose(0, 2, 5, 1, 4, 3)                  # [p, k, ch, i, d, jj]
        vb = vb.reshape(NP, KC, C, G * LC)
        maps.append(
            {
                "vin": np.ascontiguousarray(vb),
                "s0q": np.ascontiguousarray(
                    s0q[core * BL : (core + 1) * BL].T
                ),
                "wkT": wkT,
                "c8col": c8col,
                "ccol": ccol,
            }
        )
    return maps, s


def _assemble(results, s):
    """Per-core 'yq' [NP, KC, C, G*LC] uint8 -> full [B, T, C] f32."""
    out = np.empty((B, T, C), np.float32)
    for core, r in enumerate(results):
        yq = np.asarray(r["yq"]).reshape(NP, KC, C, 2, D, LC)
        af = yq.astype(np.float32) - 128.0
        # stream D-2 is written by the DVE STT as signed int8 (no +128 bias)
        s6 = np.ascontiguousarray(yq[:, :, :, :, D - 2, :]).view(np.int8)
        af[:, :, :, :, D - 2, :] = s6.astype(np.float32)
        a = af * s
        a = a.transpose(0, 3, 1, 5, 4, 2)        # [p, i, k, jj, d, ch]
        a = a.reshape(BL, L, D, C)               # t = 8*(k*LC+jj) + d
        out[core * BL : (core + 1) * BL] = a.reshape(BL, T, C)
    return out


def _ensure_ntff_hook():
    """Shim antenv.axon_hooks (absent in this image) so trace=True works."""
    import types

    import antenv

    if not hasattr(antenv, "axon_hooks"):
        mod = types.ModuleType("antenv.axon_hooks")
        holder = [None]
        mod.set_axon_ntff_profile_hook = lambda h: holder.__setitem__(0, h)
        mod.get_axon_ntff_profile_hook = lambda: holder[0]
        sys.modules["antenv.axon_hooks"] = mod
        antenv.axon_hooks = mod
    from antenv.axon_hooks import (
        get_axon_ntff_profile_hook,
        set_axon_ntff_profile_hook,
    )

    if get_axon_ntff_profile_hook() is None:
        from trn_agent_boot.trn_boot import _ntff_profile_via_ctypes

        set_axon_ntff_profile_hook(
            _ntff_profile_via_ctypes("/opt/axon/libaxon_pjrt.so")
        )


def run(inputs, initial_state, weights, trace=False, **kw):
    from concourse import bass_utils

    if trace:
        _ensure_ntff_hook()
    nc = build_bass()
    maps, s = _prep(inputs, initial_state, weights)
    res = bass_utils.run_bass_kernel_spmd(
        nc, maps, core_ids=list(range(NCORES)), trace=trace, **kw
    )
    out = _assemble(res.results, s)
    return out, res


def kernel(inputs, initial_state, weights):
    out, _ = run(inputs, initial_state, weights)
    return out


# revision 44
# speedup vs baseline: 1.0333x; 1.0333x over previous
"""EMA recurrence kernel for Trainium2 (8 NeuronCores, Bass/Tile).

Computes a_t = w * x_t + (1 - w) * a_{t-1} over inputs [B=32, T=8192, C=128],
initial_state [B, C], weights [C] -> output [B, T, C].

Strategy (v7 -- depth-8 decimated scan, Q-space, uint8 output):
  - Pure data parallelism: batch dim sharded 4-per-core across 8 cores.
  - Everything on-device runs in "Q-space": host pre-scales v = (w*x)/s
    (fp16) where s = max|a|/126, so outputs quantize to uint8 via
    trunc(Q + 128.5) -- always positive, so truncation == floor == exact
    round-half-up regardless of HW convert semantics. Output HBM traffic
    is 1/4 of fp32, input traffic 1/2.
  - Time decimated by D=8 into streams d=0..7 (t = 8j + d):
      PE    U[j] = sum_d c^{7-d} v_d[j]  (8 diag-matmul passes -> PSUM,
            one PSUM tile per batch so each scan is fenced only on its
            own 8 matmuls -- the framework fences at tile granularity)
      DVE   scan Q8[j] = c^8 Q8[j-1] + U[j], reading U directly from
            PSUM (no ACT evacuation), fp16 out
      DVE   recon chain Y_0 = c*Q8[j-1] + v_0; Y_d = c*Y_{d-1} + v_d
            as TS (4x perf mode) + TT (2x perf mode), fused over the 2
            batches of a pair; one yd tile per stream so each ACT quant
            starts right after its own TT
      ACT   quantize fp16 -> uint8 (ACT cost is dtype-independent, so
            the int8 conversion is free there)
      DMA   input quarters on the SP ring; outputs on the GPSIMD ring
            (separate rings: both are in-order, so mixing input and
            output transfers head-of-line-blocks the pipeline)
  - The recon chain stays entirely on DVE: it is latency-sensitive and
    cross-engine chain steps (ACT or GPSIMD) measured worse. GPSIMD
    compute also degrades DVE throughput via SBUF arbitration.
  - Host-side DRAM layouts are partition-row contiguous ([C, cols]) so
    every DMA is a plain 2D copy (128 descriptors, KBs each).
  - Work units = (batch pair) x (column chunk): 4 units pipeline
    DMA/PE/DVE/ACT; measured best among pair/quad x LC {512, 1024}.
"""

import sys

if "/opt/trn_rl_repo" not in sys.path:
    sys.path.insert(0, "/opt/trn_rl_repo")

import numpy as np

B, T, C = 32, 8192, 128
NCORES = 8
BL = B // NCORES      # batches per core (4)
D = 8                 # decimation depth
L = T // D            # decimated stream length (1024)
NP = BL // 2          # batch pairs per core (2)
LC = 512              # scan chunk columns
KC = L // LC          # chunks per stream (2)
G = 2 * D             # blocks per unit: (half i, stream d), i-major
MM = 512              # matmul slice (PE moving-dim limit / PSUM bank)

_NC_CACHE = None


def build_bass():
    global _NC_CACHE
    if _NC_CACHE is not None:
        return _NC_CACHE

    import concourse.bacc as bacc
    import concourse.mybir as mybir
    import concourse.tile as tile

    f32 = mybir.dt.float32
    f16 = mybir.dt.float16
    u8 = mybir.dt.uint8
    i8 = mybir.dt.int8
    AF = mybir.ActivationFunctionType
    ALU = mybir.AluOpType

    W2 = 2 * LC           # fused pair width per stream (1024)
    VW = G * LC           # full unit width (8192)
    HW = D * LC           # half-unit width (one batch, 4096)
    SL = 1 + L            # per-batch scan row incl. init col

    nc = bacc.Bacc("TRN2", target_bir_lowering=False, debug=False)
    vin = nc.dram_tensor("vin", [NP, KC, C, VW], f16, kind="ExternalInput").ap()
    s0q = nc.dram_tensor("s0q", [C, BL], f16, kind="ExternalInput").ap()
    wkT = nc.dram_tensor("wkT", [C, D * 128], f16, kind="ExternalInput").ap()
    c8col = nc.dram_tensor("c8col", [C, 1], f32, kind="ExternalInput").ap()
    ccol = nc.dram_tensor("ccol", [C, 1], f32, kind="ExternalInput").ap()
    yq = nc.dram_tensor("yq", [NP, KC, C, VW], u8, kind="ExternalOutput").ap()

    def blk(i, d):        # vt/qt column base of block (half i, stream d)
        return (i * D + d) * LC

    with tile.TileContext(nc) as tc:
        with (
            tc.tile_pool(name="const", bufs=1) as cpool,
            tc.tile_pool(name="vin", bufs=4) as vpool,
            tc.tile_pool(name="ups", bufs=4, space="PSUM") as ppool,
            tc.tile_pool(name="y8", bufs=1) as spool,
            tc.tile_pool(name="work", bufs=3) as wpool,
            tc.tile_pool(name="yout", bufs=3) as ypool,
        ):
            # consts ride the ACT ring; the v stream starts at once on SP
            wkT_t = cpool.tile([C, D * 128], f16, name="wkT_t")
            nc.scalar.dma_start(wkT_t[:], wkT[:])
            c8_t = cpool.tile([C, 1], f32, name="c8_t")
            nc.scalar.dma_start(c8_t[:], c8col[:])
            c_t = cpool.tile([C, 1], f32, name="c_t")
            nc.scalar.dma_start(c_t[:], ccol[:])

            # pair-fused scan rows: batch i of pair p at cols [i*SL, i*SL+SL);
            # init cols via one tiny strided DMA per pair
            y8t = [spool.tile([C, 2 * SL], f16, name=f"y8_{p}") for p in range(NP)]
            for p in range(NP):
                nc.scalar.dma_start(
                    y8t[p][:].rearrange("c (i e) -> c i e", i=2)[:, :, 0:1],
                    s0q[:, 2 * p : 2 * p + 2].unsqueeze(2),
                )

            for k in range(KC):
                for p in range(NP):
                    b0 = 2 * p
                    lo, hi = k * LC, (k + 1) * LC

                    # ---- input DMA: quarter transfers (half i, streams
                    # d0..3 / d4..7) on SP so PE starts on the first quarter;
                    # vpool bufs=4 covers all units so these never wait
                    vt = vpool.tile([C, VW], f16, name=f"v{p}_{k}", tag="v")
                    QW = VW // 4
                    for q in range(4):
                        nc.sync.dma_start(
                            vt[:, q * QW : (q + 1) * QW],
                            vin[p][k][:, q * QW : (q + 1) * QW],
                        )

                    # ---- PE: U_i = sum_d diag(c^{7-d}) @ v_(i,d) into a
                    # per-batch PSUM tile; scan issues right after each
                    for i in range(2):
                        upi = ppool.tile([C, LC], f32, name=f"up{i}",
                                         tag=f"up{i}")
                        for d in range(D):
                            vcol = blk(i, d)
                            nc.tensor.matmul(
                                upi[:],
                                wkT_t[:, d * 128 : (d + 1) * 128],
                                vt[:, vcol : vcol + LC],
                                start=(d == 0),
                                stop=(d == D - 1),
                            )
                        base = i * SL
                        nc.vector.tensor_tensor_scan(
                            y8t[p][:, base + 1 + lo : base + 1 + hi],
                            c8_t[:, 0:1].broadcast_to([C, LC]),
                            upi[:],
                            y8t[p][:, base + lo : base + lo + 1],
                            op0=ALU.mult,
                            op1=ALU.add,
                        )

                    # ---- recon chain on DVE (TS 4x + TT 2x), pair-fused,
                    # one yd tile per stream for tile-granular fencing
                    qt = ypool.tile([C, VW], u8, name=f"q{p}_{k}", tag="q")
                    prev = None
                    for d in range(D - 2):
                        cae = wpool.tile([C, W2], f16, name="cae", tag="cae")
                        if d == 0 and p == 0 and k == 0:
                            # first unit: per-batch TS halves so DVE can run
                            # TS(i0) in the gap while PE finishes batch i1
                            for i in range(2):
                                nc.vector.tensor_scalar(
                                    cae[:, i * LC : (i + 1) * LC],
                                    y8t[p][:, i * SL + lo : i * SL + hi],
                                    c_t[:, 0:1],
                                    None,
                                    op0=ALU.mult,
                                )
                        elif d == 0:
                            src = y8t[p][:].rearrange("c (i e) -> c i e", i=2)[
                                :, :, lo:hi
                            ]
                            nc.vector.tensor_scalar(
                                cae[:].rearrange("c (i b) -> c i b", i=2),
                                src,
                                c_t[:, 0:1],
                                None,
                                op0=ALU.mult,
                            )
                        else:
                            nc.vector.tensor_scalar(
                                cae[:], prev[:], c_t[:, 0:1], None,
                                op0=ALU.mult,
                            )
                        v3 = vt[:].rearrange("c (i g) -> c i g", i=2)[
                            :, :, d * LC : (d + 1) * LC
                        ]
                        yd = wpool.tile([C, W2], f16, name=f"yd{d}",
                                        tag=f"yd{d}")
                        nc.vector.tensor_tensor(
                            yd[:].rearrange("c (i b) -> c i b", i=2),
                            cae[:].rearrange("c (i b) -> c i b", i=2),
                            v3,
                            op=ALU.add,
                        )
                        prev = yd
                        # quantize this stream now (starts right after TT)
                        nc.scalar.activation(
                            qt[:].rearrange("c (i g) -> c i g", i=2)[
                                :, :, d * LC : (d + 1) * LC
                            ],
                            yd[:].rearrange("c (i b) -> c i b", i=2),
                            AF.Copy,
                            bias=128.5,
                            scale=1.0,
                        )

                    # scan stream (d = D-1) quantize, both halves in one op
                    nc.scalar.activation(
                        qt[:].rearrange("c (i g) -> c i g", i=2)[
                            :, :, (D - 1) * LC : D * LC
                        ],
                        y8t[p][:].rearrange("c (i e) -> c i e", i=2)[
                            :, :, 1 + lo : 1 + hi
                        ],
                        AF.Copy,
                        bias=128.5,
                        scale=1.0,
                    )

                    # stream 6: chain step + quantize fused into ONE DVE STT
                    # writing signed int8 (y_6 has no downstream consumer, so
                    # the fp16 intermediate and its ACT quant are unnecessary)
                    nc.vector.scalar_tensor_tensor(
                        qt[:].rearrange("c (i g) -> c i g", i=2)[
                            :, :, (D - 2) * LC : (D - 1) * LC
                        ].bitcast(i8),
                        prev[:].rearrange("c (i b) -> c i b", i=2),
                        c_t[:, 0:1],
                        vt[:].rearrange("c (i g) -> c i g", i=2)[
                            :, :, (D - 2) * LC : (D - 1) * LC
                        ],
                        op0=ALU.mult,
                        op1=ALU.add,
                    )


                    # ---- output DMA quarters on the GPSIMD ring
                    OW = VW // 4
                    for h in range(4):
                        nc.gpsimd.dma_start(
                            yq[p][k][:, h * OW : (h + 1) * OW],
                            qt[:, h * OW : (h + 1) * OW],
                        )

    nc.compile()
    _NC_CACHE = nc
    return nc


def _prep(inputs, initial_state, weights):
    x = np.asarray(inputs, dtype=np.float32)
    s0 = np.asarray(initial_state, dtype=np.float32)
    w = np.clip(np.asarray(weights, dtype=np.float32), 0.0, 1.0)
    c = (1.0 - w).astype(np.float32)

    M = max(np.abs(x).max(), np.abs(s0).max())
    s = np.float32(M / 126.0)

    # v[b, j, d, ch] = w * x[b, 8j+d, ch] / s   (fp16)
    v = (w[None, None, :] * x / s).astype(np.float16)        # [B, T, C]
    v = v.reshape(B, L, D, C)

    s0q = (s0 / s).astype(np.float16)                        # [B, C]

    wkT = np.zeros((C, D * 128), np.float16)
    cd = c.astype(np.float64)
    for d in range(D):
        np.fill_diagonal(
            wkT[:, d * 128 : (d + 1) * 128], (cd ** (D - 1 - d)).astype(np.float16)
        )

    c8col = np.ascontiguousarray((cd**D).astype(np.float32)[:, None])
    ccol = np.ascontiguousarray(c[:, None])

    maps = []
    for core in range(NCORES):
        vb = v[core * BL : (core + 1) * BL]                  # [BL, L, D, C]
        vb = vb.reshape(NP, 2, KC, LC, D, C)                 # [p, i, k, jj, d, ch]
        vb = vb.transpose(0, 2, 5, 1, 4, 3)                  # [p, k, ch, i, d, jj]
        vb = vb.reshape(NP, KC, C, G * LC)
        maps.append(
            {
                "vin": np.ascontiguousarray(vb),
                "s0q": np.ascontiguousarray(
                    s0q[core * BL : (core + 1) * BL].T
                ),
                "wkT": wkT,
                "c8col": c8col,
                "ccol": ccol,
            }
        )
    return maps, s


def _assemble(results, s):
    """Per-core 'yq' [NP, KC, C, G*LC] uint8 -> full [B, T, C] f32."""
    out = np.empty((B, T, C), np.float32)
    for core, r in enumerate(results):
        yq = np.asarray(r["yq"]).reshape(NP, KC, C, 2, D, LC)
        af = yq.astype(np.float32) - 128.0
        # stream D-2 is written by the DVE STT as signed int8 (no +128 bias)
        s6 = np.ascontiguousarray(yq[:, :, :, :, D - 2, :]).view(np.int8)
        af[:, :, :, :, D - 2, :] = s6.astype(np.float32)
        a = af * s
        a = a.transpose(0, 3, 1, 5, 4, 2)        # [p, i, k, jj, d, ch]
        a = a.reshape(BL, L, D, C)               # t = 8*(k*LC+jj) + d
        out[core * BL : (core + 1) * BL] = a.reshape(BL, T, C)
    return out


def _ensure_ntff_hook():
    """Shim antenv.axon_hooks (absent in this image) so trace=True works."""
    import types

    import antenv

    if not hasattr(antenv, "axon_hooks"):
        mod = types.ModuleType("antenv.axon_hooks")
        holder = [None]
        mod.set_axon_ntff_profile_hook = lambda h: holder.__setitem__(0, h)
        mod.get_axon_ntff_profile_hook = lambda: holder[0]
        sys.modules["antenv.axon_hooks"] = mod
        antenv.axon_hooks = mod
    from antenv.axon_hooks import (
        get_axon_ntff_profile_hook,
        set_axon_ntff_profile_hook,
    )

    if get_axon_ntff_profile_hook() is None:
        from trn_agent_boot.trn_boot import _ntff_profile_via_ctypes

        set_axon_ntff_profile_hook(
            _ntff_profile_via_ctypes("/opt/axon/libaxon_pjrt.so")
        )


def run(inputs, initial_state, weights, trace=False, **kw):
    from concourse import bass_utils

    if trace:
        _ensure_ntff_hook()
    nc = build_bass()
    maps, s = _prep(inputs, initial_state, weights)
    res = bass_utils.run_bass_kernel_spmd(
        nc, maps, core_ids=list(range(NCORES)), trace=trace, **kw
    )
    out = _assemble(res.results, s)
    return out, res


def kernel(inputs, initial_state, weights):
    out, _ = run(inputs, initial_state, weights)
    return out
